# revision 20
# baseline (speedup 1.0000x reference)
"""DeepseekMoE Trainium2 kernel (8-core SPMD, expert MoE + shared expert).

Math (faithful to the reference quirk):
  logits = x @ gate_w.T                     [T, 8]
  top2 per token -> (sel1, sel2), softmax over the two logits -> (r1, r2)
  s_e = sum over ALL tokens of per-token routing weight mass for expert e
  out[t] = sum_{e in top2(t)} s_e * MLP_e(x_t) + shared_mlp(x_t)

Sharding: token-parallel. Core i owns tokens [i*1024, (i+1)*1024); it computes
the gate for its tokens in fp32 on the PE, the per-expert weight-mass partial
sums, AllReduces them (32 B) to get the global s_e, then runs all 8 expert
MLPs (bf16 matmuls, fp32 accumulate) dense-masked over its tokens plus the
shared expert. Output is the core's [2048, 1024] feature-major slice; host
concatenates and transposes.
"""

import sys
from contextlib import ExitStack

import numpy as np

sys.path.insert(0, "/opt/trn_rl_repo")

import ml_dtypes

import concourse.bacc as bacc
import concourse.mybir as mybir
import concourse.tile as tile
from concourse import masks
from concourse._compat import with_exitstack
from concourse.bass_utils import run_bass_kernel_spmd

F32 = mybir.dt.float32
BF16 = mybir.dt.bfloat16
AX = mybir.AxisListType.X
OP = mybir.AluOpType
ACT_F = mybir.ActivationFunctionType

B, S, D = 4, 2048, 2048
T = B * S              # 8192 tokens
E = 8                  # experts
H = 1408               # expert intermediate
SH = 2 * H             # shared intermediate
NCORES = 8
TLOC = T // NCORES     # 1024 tokens per core
DT = D // 128          # 16 d-tiles
HT = H // 128          # 11 h-tiles
SHT = SH // 128        # 22 shared h-tiles
HBLK = 4               # h-tiles per stage-1 weight block

_CACHE = {}


@with_exitstack
def _moe_kernel(ctx: ExitStack, tc: tile.TileContext, ins, outs):
    nc = tc.nc
    xg, xb, gwt = ins["xg"], ins["xb"], ins["gwt"]
    w1t, w3t, w2t = ins["w1t"], ins["w3t"], ins["w2t"]
    sw1t, sw3t, sw2t = ins["sw1t"], ins["sw3t"], ins["sw2t"]
    outT = outs["outT"]

    const = ctx.enter_context(tc.tile_pool(name="const", bufs=1))
    xbp = ctx.enter_context(tc.tile_pool(name="xbp", bufs=1))
    yacc = ctx.enter_context(tc.tile_pool(name="yacc", bufs=1))
    apool = ctx.enter_context(tc.tile_pool(name="apool", bufs=12))
    wst = ctx.enter_context(tc.tile_pool(name="wst", bufs=17))
    w2st = ctx.enter_context(tc.tile_pool(name="w2st", bufs=2))
    gp = ctx.enter_context(tc.tile_pool(name="gp", bufs=4))
    rt = ctx.enter_context(tc.tile_pool(name="rt", bufs=1))
    tmp = ctx.enter_context(tc.tile_pool(name="tmp", bufs=2))
    dram = ctx.enter_context(tc.tile_pool(name="dram", bufs=1, space="DRAM"))
    ps1 = ctx.enter_context(tc.tile_pool(name="ps1", bufs=2, space="PSUM"))
    ps2 = ctx.enter_context(tc.tile_pool(name="ps2", bufs=2, space="PSUM"))
    psr = ctx.enter_context(tc.tile_pool(name="psr", bufs=1, space="PSUM"))

    ident = const.tile([128, 128], F32)
    masks.make_identity(nc, ident[:])
    ones_col = const.tile([128, 1], F32)
    nc.vector.memset(ones_col[:], 1.0)
    ones_row = const.tile([1, 128], F32)
    nc.vector.memset(ones_row[:], 1.0)

    # gate weights: gwt DRAM [2048, 8] -> sbuf [128, 16*8]
    gwt_sb = const.tile([128, DT * E], F32)
    gwt_v = gwt.rearrange("(dt p) e -> dt p e", p=128)
    for d in range(DT):
        nc.sync.dma_start(gwt_sb[:, d * E:(d + 1) * E], gwt_v[d])

    # resident bf16 activations: xb DRAM [2048, 1024] -> [128, 16*1024]
    xb_v = xb.rearrange("(dt p) t -> dt p t", p=128)
    xb_sb = xbp.tile([128, DT * TLOC], BF16)
    for d in range(DT):
        nc.sync.dma_start(xb_sb[:, d * TLOC:(d + 1) * TLOC], xb_v[d])

    def xbt(d):
        return xb_sb[:, d * TLOC:(d + 1) * TLOC]

    # ---------------- gate (fp32) + routing ----------------
    xg_v = xg.rearrange("(dt p) t -> dt p t", p=128)
    LTs = []
    for c in range(2):  # 512-token chunks
        psl = psr.tile([8, 512], F32, tag="r")
        for d in range(DT):
            gx = gp.tile([128, 512], F32, tag="gx")
            nc.sync.dma_start(gx[:], xg_v[d][:, c * 512:(c + 1) * 512])
            nc.tensor.matmul(psl[:], gwt_sb[:, d * E:(d + 1) * E], gx[:],
                             start=(d == 0), stop=(d == DT - 1))
        lg = rt.tile([8, 512], F32, tag=f"lg{c}")
        nc.scalar.activation(lg[:], psl[:], ACT_F.Copy)
        for q in range(4):
            pst = psr.tile([128, 8], F32, tag="r")
            nc.tensor.transpose(pst[:], lg[:, q * 128:(q + 1) * 128],
                                ident[0:8, 0:8])
            LT = rt.tile([128, 8], F32, tag=f"LT{c * 4 + q}")
            nc.vector.tensor_copy(LT[:], pst[:])
            LTs.append(LT)

    sW = rt.tile([128, 8], F32)
    nc.vector.memset(sW[:], 0.0)
    mask_ts = []
    for i, LT in enumerate(LTs):
        m1 = rt.tile([128, 1], F32, tag="m1")
        nc.vector.reduce_max(m1[:], LT[:], axis=AX)
        is1 = rt.tile([128, 8], F32, tag=f"is1_{i}")
        nc.vector.tensor_scalar(is1[:], LT[:], m1[:], None, OP.is_equal)
        big = rt.tile([128, 8], F32, tag="big")
        nc.vector.tensor_scalar(big[:], is1[:], 1e30, None, OP.mult)
        Lm = rt.tile([128, 8], F32, tag="Lm")
        nc.vector.tensor_tensor(Lm[:], LT[:], big[:], OP.subtract)
        m2 = rt.tile([128, 1], F32, tag="m2")
        nc.vector.reduce_max(m2[:], Lm[:], axis=AX)
        is2 = rt.tile([128, 8], F32, tag=f"is2_{i}")
        nc.vector.tensor_scalar(is2[:], Lm[:], m2[:], None, OP.is_equal)
        d12 = rt.tile([128, 1], F32, tag="d12")
        nc.vector.tensor_tensor(d12[:], m1[:], m2[:], OP.subtract)
        r1 = rt.tile([128, 1], F32, tag="r1")
        nc.scalar.activation(r1[:], d12[:], ACT_F.Sigmoid)
        r2 = rt.tile([128, 1], F32, tag="r2")
        nc.vector.tensor_scalar(r2[:], r1[:], -1.0, 1.0, OP.mult, OP.add)
        w1c = rt.tile([128, 8], F32, tag="w1c")
        nc.vector.tensor_scalar(w1c[:], is1[:], r1[:], None, OP.mult)
        w2c = rt.tile([128, 8], F32, tag="w2c")
        nc.vector.tensor_scalar(w2c[:], is2[:], r2[:], None, OP.mult)
        Wt = rt.tile([128, 8], F32, tag="Wt")
        nc.vector.tensor_tensor(Wt[:], w1c[:], w2c[:], OP.add)
        nc.vector.tensor_tensor(sW[:], sW[:], Wt[:], OP.add)
        mask = rt.tile([128, 8], F32, tag=f"mask{i}")
        nc.vector.tensor_tensor(mask[:], is1[:], is2[:], OP.add)
        mask_ts.append(mask)

    # s_partial = ones_col.T @ sW -> [1, 8]; AllReduce across cores
    pss = psr.tile([1, 8], F32, tag="r")
    nc.tensor.matmul(pss[:], ones_col[:], sW[:], start=True, stop=True)
    sp_sb = rt.tile([1, 8], F32)
    nc.vector.tensor_copy(sp_sb[:], pss[:])
    cc_in = dram.tile([1, 8], F32)
    cc_out = dram.tile([1, 8], F32, addr_space="Shared")
    nc.sync.dma_start(cc_in[:], sp_sb[:])
    nc.gpsimd.collective_compute(
        "AllReduce", OP.add, replica_groups=[list(range(NCORES))],
        ins=[cc_in[:]], outs=[cc_out[:]],
    )
    s_row = rt.tile([1, 8], F32)
    nc.sync.dma_start(s_row[:], cc_out[:])

    # maskT [8, 1024] via per-tile PE transposes
    maskT = rt.tile([8, TLOC], F32)
    for i in range(8):
        psst = psr.tile([8, 128], F32, tag="r")
        nc.tensor.transpose(psst[:], mask_ts[i][:], ident[:])
        nc.vector.tensor_copy(maskT[:, i * 128:(i + 1) * 128], psst[:])
    # s as a column [8, 1] (SBUF->SBUF DMA moves across partitions)
    s_col = rt.tile([8, 1], F32)
    nc.sync.dma_start(s_col[:, 0:1], s_row[0:1, :])
    # scaleT_full[e, t] = mask[e, t] * s_e
    scaleT_full = rt.tile([8, TLOC], F32)
    nc.vector.tensor_scalar(scaleT_full[:], maskT[:], s_col[:], None, OP.mult)
    # sel8[k, e*128+p] = [k == e]: row-select matrices (host-provided const)
    sel8 = const.tile([8, E * 128], F32)
    nc.sync.dma_start(sel8[:], ins["sel8"][:])

    # ---------------- expert MLPs (bf16) ----------------
    ya = yacc.tile([128, DT * TLOC], F32)

    def yat(d):
        return ya[:, d * TLOC:(d + 1) * TLOC]

    def stage1(n_ht, w1_ap, w3_ap, scale_rep):
        """Produces n_ht a-tiles [128, TLOC] bf16."""
        a_tiles = []
        for h0 in range(0, n_ht, HBLK):
            hn = min(HBLK, n_ht - h0)
            w1tl, w3tl = [], []
            for d in range(DT):
                wt1 = wst.tile([128, HBLK * 128], BF16, tag="wt1")
                nc.sync.dma_start(
                    wt1[:, :hn * 128],
                    w1_ap[d * 128:(d + 1) * 128, h0 * 128:(h0 + hn) * 128])
                wt3 = wst.tile([128, HBLK * 128], BF16, tag="wt3")
                nc.sync.dma_start(
                    wt3[:, :hn * 128],
                    w3_ap[d * 128:(d + 1) * 128, h0 * 128:(h0 + hn) * 128])
                w1tl.append(wt1)
                w3tl.append(wt3)
            for hb in range(hn):
                hs = slice(hb * 128, (hb + 1) * 128)
                a_t = apool.tile([128, TLOC], BF16, tag="a")
                for c in range(2):
                    cs = slice(c * 512, (c + 1) * 512)
                    pg = ps1.tile([128, 512], F32, tag="pg")
                    pu = ps1.tile([128, 512], F32, tag="pu")
                    for d in range(DT):
                        nc.tensor.matmul(pg[:], w1tl[d][:, hs], xbt(d)[:, cs],
                                         start=(d == 0), stop=(d == DT - 1))
                    for d in range(DT):
                        nc.tensor.matmul(pu[:], w3tl[d][:, hs], xbt(d)[:, cs],
                                         start=(d == 0), stop=(d == DT - 1))
                    sg = tmp.tile([128, 512], F32, tag="sg")
                    nc.scalar.activation(sg[:], pg[:], ACT_F.Sigmoid)
                    sil = tmp.tile([128, 512], F32, tag="sil")
                    nc.vector.tensor_tensor(sil[:], pg[:], sg[:], OP.mult)
                    if scale_rep is not None:
                        us = tmp.tile([128, 512], F32, tag="us")
                        nc.vector.tensor_tensor(us[:], pu[:],
                                                scale_rep[:, cs], OP.mult)
                        nc.vector.tensor_tensor(a_t[:, cs], sil[:], us[:],
                                                OP.mult)
                    else:
                        nc.vector.tensor_tensor(a_t[:, cs], sil[:], pu[:],
                                                OP.mult)
                a_tiles.append(a_t)
        return a_tiles

    def stage2(n_ht, w2blk_src, a_tiles, first):
        for d in range(DT):
            w2blk = w2st.tile([128, n_ht * 128], BF16, tag="w2blk")
            nc.sync.dma_start(w2blk[:], w2blk_src(d))
            for c in range(2):
                cs = slice(c * 512, (c + 1) * 512)
                py = ps2.tile([128, 512], F32, tag="py")
                for hb in range(n_ht):
                    nc.tensor.matmul(py[:],
                                     w2blk[:, hb * 128:(hb + 1) * 128],
                                     a_tiles[hb][:, cs],
                                     start=(hb == 0), stop=(hb == n_ht - 1))
                if first:
                    nc.vector.tensor_copy(yat(d)[:, cs], py[:])
                else:
                    nc.vector.tensor_tensor(yat(d)[:, cs], yat(d)[:, cs],
                                            py[:], OP.add)

    for e in range(E):
        # scale_rep[p, t] = scale[t, e] replicated over all partitions
        psc = psr.tile([128, TLOC], F32, tag="r")
        for c in range(2):
            cs = slice(c * 512, (c + 1) * 512)
            nc.tensor.matmul(psc[:, cs], sel8[:, e * 128:(e + 1) * 128],
                             scaleT_full[:, cs], start=True, stop=True)
        scale_rep = tmp.tile([128, TLOC], F32, tag="screp")
        nc.vector.tensor_copy(scale_rep[:], psc[:])

        a_tiles = stage1(HT, w1t[e], w3t[e], scale_rep)
        stage2(HT, lambda d, e=e: w2t[e, d], a_tiles, first=(e == 0))

    # shared expert: two half passes of 11 h-tiles
    for hp in range(2):
        r0 = hp * HT * 128
        a_tiles = stage1(HT, sw1t[:, r0:r0 + HT * 128],
                         sw3t[:, r0:r0 + HT * 128], None)
        stage2(HT,
               lambda d, hp=hp: sw2t[d][:, hp * HT * 128:(hp + 1) * HT * 128],
               a_tiles, first=False)

    # store
    outT_v = outT.rearrange("(dt p) t -> dt p t", p=128)
    for d in range(DT):
        nc.sync.dma_start(outT_v[d], yat(d))


def _declare(nc):
    ins = {
        "xg": nc.dram_tensor("xg", [D, TLOC], F32, kind="ExternalInput").ap(),
        "xb": nc.dram_tensor("xb", [D, TLOC], BF16, kind="ExternalInput").ap(),
        "gwt": nc.dram_tensor("gwt", [D, E], F32, kind="ExternalInput").ap(),
        "w1t": nc.dram_tensor("w1t", [E, D, H], BF16, kind="ExternalInput").ap(),
        "w3t": nc.dram_tensor("w3t", [E, D, H], BF16, kind="ExternalInput").ap(),
        "w2t": nc.dram_tensor("w2t", [E, DT, 128, H], BF16,
                              kind="ExternalInput").ap(),
        "sw1t": nc.dram_tensor("sw1t", [D, SH], BF16, kind="ExternalInput").ap(),
        "sw3t": nc.dram_tensor("sw3t", [D, SH], BF16, kind="ExternalInput").ap(),
        "sw2t": nc.dram_tensor("sw2t", [DT, 128, SH], BF16,
                               kind="ExternalInput").ap(),
        "sel8": nc.dram_tensor("sel8", [E, E * 128], F32,
                               kind="ExternalInput").ap(),
    }
    outs = {
        "outT": nc.dram_tensor("outT", [D, TLOC], F32,
                               kind="ExternalOutput").ap(),
    }
    return ins, outs


def _build():
    if "nc" in _CACHE:
        return _CACHE["nc"]
    nc = bacc.Bacc("TRN2", target_bir_lowering=False, debug=False,
                   num_devices=NCORES)
    ins, outs = _declare(nc)
    with tile.TileContext(nc, trace_sim=False) as tc:
        _moe_kernel(tc, ins, outs)
    nc.compile()
    _CACHE["nc"] = nc
    return nc


def _prep_inputs(hidden_states, gate_w, w1, w3, w2, sw1, sw3, sw2):
    if "weights" not in _CACHE:
        bf = ml_dtypes.bfloat16
        _CACHE["weights"] = {
            "gwt": np.ascontiguousarray(np.asarray(gate_w, np.float32).T),
            "w1t": np.ascontiguousarray(
                np.asarray(w1, np.float32).transpose(0, 2, 1)).astype(bf),
            "w3t": np.ascontiguousarray(
                np.asarray(w3, np.float32).transpose(0, 2, 1)).astype(bf),
            # w2t[e, dtile, p, ht*128+dd] = w2[e, dtile*128+dd, ht*128+p]
            # i.e. lhsT blocks [128h(K) x 128d(M)] packed contiguously per
            # (e, dtile) so the stage2 block load is one clean 2D DMA.
            "w2t": np.ascontiguousarray(
                np.asarray(w2, np.float32)
                .reshape(E, DT, 128, HT, 128)      # [e, dt, dd, ht, p]
                .transpose(0, 1, 4, 3, 2)          # [e, dt, p, ht, dd]
                .reshape(E, DT, 128, H)).astype(bf),
            "sw1t": np.ascontiguousarray(np.asarray(sw1, np.float32).T).astype(bf),
            "sw3t": np.ascontiguousarray(np.asarray(sw3, np.float32).T).astype(bf),
            "sw2t": np.ascontiguousarray(
                np.asarray(sw2, np.float32)
                .reshape(DT, 128, SHT, 128)        # [dt, dd, ht, p]
                .transpose(0, 3, 2, 1)             # [dt, p, ht, dd]
                .reshape(DT, 128, SH)).astype(bf),
            "sel8": np.repeat(np.eye(E, dtype=np.float32), 128,
                              axis=1).reshape(E, E * 128),
        }
    wts = _CACHE["weights"]
    x = np.asarray(hidden_states, dtype=np.float32).reshape(T, D)
    in_maps = []
    for i in range(NCORES):
        xs = np.ascontiguousarray(x[i * TLOC:(i + 1) * TLOC].T)
        in_maps.append({"xg": xs, "xb": xs.astype(ml_dtypes.bfloat16), **wts})
    return in_maps


def kernel(hidden_states, gate_w, w1, w3, w2, sw1, sw3, sw2,
           _trace=False, _result_box=None):
    nc = _build()
    in_maps = _prep_inputs(hidden_states, gate_w, w1, w3, w2, sw1, sw3, sw2)
    res = run_bass_kernel_spmd(nc, in_maps, list(range(NCORES)), trace=_trace)
    if _result_box is not None:
        _result_box.append(res)
    out = np.empty((T, D), dtype=np.float32)
    for i in range(NCORES):
        out[i * TLOC:(i + 1) * TLOC] = res.results[i]["outT"].T
    return out.reshape(B, S, D)
